# revision 1
# baseline (speedup 1.0000x reference)
"""NNUE-style embedding-lookup + tiny-MLP kernel for Trainium2 (8 NeuronCores).

Data-parallel over the batch dim: each of the 8 cores handles 2048 of the
16384 batch positions; the 50 MB embedding table and MLP weights are
replicated per core.

Per-core device program:
  1. dma_gather (SWDGE batched gather) pulls the 32 active-feature embedding
     rows per batch position from HBM into SBUF, 2064 rows per call, rotated
     across 4 SWDGE queues so descriptor generation runs on all Q7 core
     pairs concurrently. Indices are pre-biased by -16384 so they fit the
     gather's signed-int16 index format (the Q7 descriptor generator
     sign-extends and does a signed multiply-accumulate onto the base
     address, so the base is advanced by +16384 rows).
  2. TensorE accumulates the gathered slots (one feature x 128 batches each)
     into PSUM with float32r identity matmuls, two slots (512 columns) per
     matmul; a DVE add folds the even/odd halves into x[128b, 256].
  3. TensorE transposes x, then runs the 256->32->32->1 MLP with ScalarE
     handling bias+ReLU/Tanh.
"""

import numpy as np

INPUT_DIM = 49152
E = 256              # embedding width (1 KB rows)
BATCH = 16384
F = 32               # active features per position
N_CORES = 8
B_CORE = BATCH // N_CORES          # 2048 batch positions per core
BIAS = 16384                       # index bias for int16 gather
CHUNK_REAL = 2048                  # gathered rows per call (16 feats x 128 batches)
PAD = 16                           # tail pad (keeps last index >= 0)
CHUNK = CHUNK_REAL + PAD           # 2064
CW = CHUNK // 16                   # 129 idx cols per chunk in [16, .] layout
CW_STRIDE = 160                    # padded col stride: 320 B, 64 B-aligned
NCH = (B_CORE * F) // CHUNK_REAL   # 32 full-size gather calls (before tail split)
NT = B_CORE // 128                 # 16 t-blocks (128 batches each)
S_REAL = CHUNK_REAL // 128         # 16 real slots per full call
S = (CHUNK + 127) // 128           # 17 slots incl. the pad slot


def _call_table():
    """Per-t-block gather calls: (col_offset, n_cols_used, num_idxs, n_real_slots).

    t-blocks 0..NT-2 use two 2048-row calls; the last t-block is split into
    four 512-row calls so the tail data-drain and compute pipeline finely.
    """
    calls = []
    col = 0
    for t in range(NT):
        sizes = [CHUNK_REAL] if CHUNK_REAL == 4096 else [CHUNK_REAL, CHUNK_REAL]
        tcalls = []
        for sz in sizes:
            n_idx = sz + PAD
            cw = n_idx // 16
            stride = ((cw + 31) // 32) * 32  # 64 B-aligned call starts
            tcalls.append((col, cw, n_idx, sz // 128))
            col += stride
        calls.append(tcalls)
    return calls, col


CALLS, IDX_COLS = _call_table()

_nc_cache = None


def _build():
    import os
    import concourse.bacc as bacc
    import concourse.mybir as mybir
    import concourse.tile as tile

    stage = os.environ.get("KERNEL_STAGE", "full")
    nt = int(os.environ.get("KERNEL_NT", str(NT)))
    bf16 = os.environ.get("KERNEL_BF16", "0") == "1"

    f32 = mybir.dt.float32
    f32r = mybir.dt.float32r
    i16 = mybir.dt.int16
    AF = mybir.ActivationFunctionType

    gdt = mybir.dt.bfloat16 if bf16 else f32r
    edt = mybir.dt.bfloat16 if bf16 else f32
    nc = bacc.Bacc(None, target_bir_lowering=False, debug=False, num_swdge_queues=4)
    emb = nc.dram_tensor("emb", [INPUT_DIM, E], edt, kind="ExternalInput")
    idx = nc.dram_tensor("idx", [128, IDX_COLS], i16, kind="ExternalInput")
    ident = nc.dram_tensor("ident", [128, 128], f32, kind="ExternalInput")
    w1t = nc.dram_tensor("w1t", [128, 2, 32], f32, kind="ExternalInput")
    b1 = nc.dram_tensor("b1", [32, 1], f32, kind="ExternalInput")
    w2l = nc.dram_tensor("w2l", [32, 32], f32, kind="ExternalInput")
    b2 = nc.dram_tensor("b2", [32, 1], f32, kind="ExternalInput")
    w3l = nc.dram_tensor("w3l", [32, 1], f32, kind="ExternalInput")
    b3 = nc.dram_tensor("b3", [1, 1], f32, kind="ExternalInput")
    out = nc.dram_tensor("out", [1, B_CORE], f32, kind="ExternalOutput")

    with tile.TileContext(nc) as tc:
        with (
            tc.tile_pool(name="const", bufs=1) as cpool,
            tc.tile_pool(name="g", bufs=(8 if CHUNK_REAL == 2048 else 5)) as gpool,
            tc.tile_pool(name="xs", bufs=2) as xspool,
            tc.tile_pool(name="xts", bufs=2) as xtspool,
            tc.tile_pool(name="hs", bufs=4) as hspool,
            tc.tile_pool(name="xp", bufs=2, space="PSUM") as xppool,
            tc.tile_pool(name="xtp", bufs=2, space="PSUM") as xtppool,
            tc.tile_pool(name="mp", bufs=4, space="PSUM") as mppool,
        ):
            idx_t = cpool.tile([128, IDX_COLS], i16)
            idx_slice = (IDX_COLS // 8 + 31) // 32 * 32
            for k in range(8):
                lo = k * idx_slice
                hi = min((k + 1) * idx_slice, IDX_COLS)
                if lo < hi:
                    nc.sync.dma_start(idx_t[:, lo:hi], idx[:, lo:hi])
            id_t = cpool.tile([128, 128], f32)
            nc.sync.dma_start(id_t[:], ident[:])
            idr_t = cpool.tile([128, 128], gdt)
            if bf16:
                nc.gpsimd.dma_start(idr_t[:], ident[:])
            else:
                nc.sync.dma_start(idr_t[:], ident[:].bitcast(f32r))
            w1t_t = cpool.tile([128, 2, 32], f32)
            nc.sync.dma_start(w1t_t[:], w1t[:])
            b1_t = cpool.tile([32, 1], f32)
            nc.sync.dma_start(b1_t[:], b1[:])
            w2l_t = cpool.tile([32, 32], f32)
            nc.sync.dma_start(w2l_t[:], w2l[:])
            b2_t = cpool.tile([32, 1], f32)
            nc.sync.dma_start(b2_t[:], b2[:])
            w3l_t = cpool.tile([32, 1], f32)
            nc.sync.dma_start(w3l_t[:], w3l[:])
            b3_t = cpool.tile([1, 1], f32)
            nc.sync.dma_start(b3_t[:], b3[:])
            out_t = cpool.tile([1, B_CORE], f32)

            qn = 0
            for t in range(nt):
                xp = xppool.tile([128, 2, E], f32, tag="xp")
                tcalls = CALLS[t]
                n_mm = sum(nrs // 2 for _, _, _, nrs in tcalls)
                mm = 0
                for col, cw, n_idx, n_real_slots in tcalls:
                    n_slots = (n_idx + 127) // 128
                    g = gpool.tile([128, S, E], gdt, tag="g")
                    nc.gpsimd.dma_gather(
                        g[:, :n_slots, :],
                        emb[BIAS:, :].bitcast(gdt),
                        idx_t[:, col : col + cw],
                        n_idx,
                        n_idx,
                        E,
                        single_packet=False,
                        queue_num=qn % 4,
                    )
                    qn += 1
                    if stage == "gather":
                        nc.sync.dma_start(
                            out[:, (qn % 8) * 128 : (qn % 8) * 128 + 128],
                            g[0:1, 0, :128].bitcast(f32),
                        )
                        continue
                    for sp in range(n_real_slots // 2):
                        nc.tensor.matmul(
                            xp[:],
                            lhsT=idr_t[:],
                            rhs=g[:, 2 * sp : 2 * sp + 2, :],
                            start=(mm == 0),
                            stop=(mm == n_mm - 1),
                        )
                        mm += 1
                if stage == "gather":
                    continue
                x_sb = xspool.tile([128, E], f32, tag="xs")
                nc.vector.tensor_reduce(
                    out=x_sb[:],
                    in_=xp[:].rearrange("p h e -> p e h"),
                    axis=mybir.AxisListType.X,
                    op=mybir.AluOpType.add,
                )
                if stage == "reduce":
                    nc.sync.dma_start(
                        out[:, (t % 8) * 256 : (t % 8) * 256 + 256], x_sb[0:1, :]
                    )
                    continue
                xt_p = xtppool.tile([128, 2, 128], f32, tag="xtp")
                for h in range(2):
                    nc.tensor.transpose(
                        xt_p[:, h, :], x_sb[:, 128 * h : 128 * (h + 1)], id_t[:]
                    )
                xt_sb = xtspool.tile([128, 2, 128], f32, tag="xts")
                nc.vector.tensor_copy(xt_sb[:], xt_p[:])
                h1p = mppool.tile([32, 128], f32, tag="mp")
                for h in range(2):
                    nc.tensor.matmul(
                        h1p[:],
                        lhsT=w1t_t[:, h, :],
                        rhs=xt_sb[:, h, :],
                        start=(h == 0),
                        stop=(h == 1),
                    )
                h1s = hspool.tile([32, 128], f32, tag="hs")
                nc.scalar.activation(h1s[:], h1p[:], AF.Relu, bias=b1_t[:])
                h2p = mppool.tile([32, 128], f32, tag="mp")
                nc.tensor.matmul(h2p[:], lhsT=w2l_t[:], rhs=h1s[:], start=True, stop=True)
                h2s = hspool.tile([32, 128], f32, tag="hs")
                nc.scalar.activation(h2s[:], h2p[:], AF.Relu, bias=b2_t[:])
                yp = mppool.tile([1, 128], f32, tag="mp")
                nc.tensor.matmul(yp[:], lhsT=w3l_t[:], rhs=h2s[:], start=True, stop=True)
                nc.scalar.activation(
                    out_t[:, 128 * t : 128 * (t + 1)], yp[:], AF.Tanh, bias=b3_t[:]
                )
            if stage == "full":
                nc.sync.dma_start(out[:], out_t[:])
    nc.compile()
    return nc


def _get_nc():
    global _nc_cache
    if _nc_cache is None:
        _nc_cache = _build()
    return _nc_cache


def _prep_indices(shard: np.ndarray) -> np.ndarray:
    """[F, B_CORE] int -> [128, NCH*CW_STRIDE] int16 device layout.

    Position order p = t*4096 + f*128 + (b % 128), t = b // 128: each gather
    slot (128 consecutive positions) holds one feature for 128 batches, so
    the feature-sum is a PSUM accumulation over the slots. Each 2064-index
    gather call covers 16 features; indices are biased by -BIAS, padded with
    16 zeros (row BIAS, keeps the tail non-negative so the Q7 truncation
    loop is a no-op), laid out [16, CW] wrapped, replicated across the 8
    Q7 core groups, and 64 B-aligned per call.
    """
    arr = shard.reshape(F, NT, 128)  # [f, t, b_in]
    biased = arr.transpose(1, 0, 2).astype(np.int64) - BIAS  # [t, f, b_in]
    outa = np.zeros((128, IDX_COLS), np.int16)
    for t in range(NT):
        flat = biased[t].reshape(-1)  # 4096 positions, feature-major
        pos = 0
        for col, cw, n_idx, n_real_slots in CALLS[t]:
            n_real = n_real_slots * 128
            lst = np.zeros(n_idx, np.int64)
            lst[:n_real] = flat[pos : pos + n_real]
            pos += n_real
            lay = lst.reshape(cw, 16).T  # [16, cw]
            outa[:, col : col + cw] = np.tile(lay, (8, 1))
    return outa


def kernel(**inputs) -> np.ndarray:
    import os
    from concourse.bass_utils import run_bass_kernel_spmd

    indices = np.asarray(inputs["indices"])
    emb = np.ascontiguousarray(np.asarray(inputs["emb"], dtype=np.float32))
    if os.environ.get("KERNEL_BF16", "0") == "1":
        import ml_dtypes

        emb = emb.astype(ml_dtypes.bfloat16)
    w1 = np.asarray(inputs["w1"], dtype=np.float32)
    b1 = np.asarray(inputs["b1"], dtype=np.float32)
    w2 = np.asarray(inputs["w2"], dtype=np.float32)
    b2 = np.asarray(inputs["b2"], dtype=np.float32)
    w3 = np.asarray(inputs["w3"], dtype=np.float32)
    b3 = np.asarray(inputs["b3"], dtype=np.float32)

    ident = np.eye(128, dtype=np.float32)
    w1t_dev = np.ascontiguousarray(w1.T.reshape(2, 128, 32).transpose(1, 0, 2))
    common = {
        "emb": emb,
        "ident": ident,
        "w1t": w1t_dev,
        "b1": b1.reshape(32, 1),
        "w2l": np.ascontiguousarray(w2.T),
        "b2": b2.reshape(32, 1),
        "w3l": np.ascontiguousarray(w3.T),
        "b3": b3.reshape(1, 1),
    }
    in_maps = []
    for c in range(N_CORES):
        shard = indices[:, c * B_CORE : (c + 1) * B_CORE]
        in_maps.append({**common, "idx": _prep_indices(shard)})

    nc = _get_nc()
    res = run_bass_kernel_spmd(nc, in_maps, core_ids=list(range(N_CORES)))
    ys = [np.asarray(res.results[c]["out"]).reshape(B_CORE) for c in range(N_CORES)]
    return np.concatenate(ys).reshape(BATCH, 1).astype(np.float32)



# revision 3
# speedup vs baseline: 1.0495x; 1.0495x over previous
"""NNUE embedding-lookup + tiny-MLP kernel for Trainium2 (8 NeuronCores).

Data-parallel over batch: each core handles 2048 of the 16384 positions; the
embedding table (converted to bf16 on host: 25 MB) and MLP weights are
replicated per core.

v2 design vs v1:
  - emb gathered in bf16: 512 B rows, exactly the SDMA line-rate floor, so
    gather HBM traffic halves to ~33.5 MB/core (~94 us at 358 GB/s).
  - feature-sum moved off TensorE: a bf16 tensor_tensor add-tree on DVE
    (2x_1P mode, 2 results/cycle) replaces the f32r identity-matmul
    accumulate whose N-cycles law floored TensorE at ~91 us/core.
  - TensorE only transposes x (bf16) and runs the 256->32->32->1 MLP.
  - PSUM->SBUF copy of x^T moved to ScalarE to keep DVE on the tree.

Per t-block (128 batches):
  2x dma_gather (2064 idx: 16 features x 128 batches + 16 pad) -> g tiles
  [128, 17, 256] bf16; tree: a=g0+g1, b=a0:8+a8:16, c, d, x [128,256] bf16;
  2x TensorE transpose -> x^T PSUM; ScalarE copy -> SBUF; W1/W2/W3 matmuls
  with ScalarE bias+ReLU/Tanh -> out[1, 128].
"""

import numpy as np

INPUT_DIM = 49152
E = 256
BATCH = 16384
F = 32
N_CORES = 8
B_CORE = BATCH // N_CORES          # 2048
BIAS = 16384                       # index bias for int16 gather
CHUNK_REAL = 2048                  # real rows per gather call (16 feat x 128 b)
PAD = 16                           # trailing pad (keeps last index >= 0)
CHUNK = CHUNK_REAL + PAD           # 2064
CW = CHUNK // 16                   # 129 idx cols per call
CW_STRIDE = 160                    # padded col stride (320 B, 64 B aligned)
NT = B_CORE // 128                 # 16 t-blocks
NCALL = 2 * NT                     # 32 gather calls
IDX_COLS = NCALL * CW_STRIDE       # 5120
S = (CHUNK + 127) // 128           # 17 slots incl pad slot

_nc_cache = None


def _build():
    import concourse.bacc as bacc
    import concourse.mybir as mybir
    import concourse.tile as tile

    f32 = mybir.dt.float32
    bf16 = mybir.dt.bfloat16
    i16 = mybir.dt.int16
    AF = mybir.ActivationFunctionType
    ADD = mybir.AluOpType.add

    nc = bacc.Bacc(
        None,
        target_bir_lowering=False,
        debug=False,
        num_swdge_queues=4,
        dynamic_dma_scratch_size=49152,
    )
    emb = nc.dram_tensor("emb", [INPUT_DIM, E], bf16, kind="ExternalInput")
    idx = nc.dram_tensor("idx", [128, IDX_COLS], i16, kind="ExternalInput")
    identb = nc.dram_tensor("identb", [128, 128], bf16, kind="ExternalInput")
    w1t = nc.dram_tensor("w1t", [128, 2, 32], f32, kind="ExternalInput")
    b1 = nc.dram_tensor("b1", [32, 1], f32, kind="ExternalInput")
    w2l = nc.dram_tensor("w2l", [32, 32], f32, kind="ExternalInput")
    b2 = nc.dram_tensor("b2", [32, 1], f32, kind="ExternalInput")
    w3l = nc.dram_tensor("w3l", [32, 1], f32, kind="ExternalInput")
    b3 = nc.dram_tensor("b3", [1, 1], f32, kind="ExternalInput")
    out = nc.dram_tensor("out", [1, B_CORE], f32, kind="ExternalOutput")

    with tile.TileContext(nc) as tc:
        with (
            tc.tile_pool(name="const", bufs=1) as cpool,
            tc.tile_pool(name="g", bufs=10) as gpool,
            tc.tile_pool(name="tb", bufs=4) as tbpool,
            tc.tile_pool(name="tc_", bufs=2) as tcpool,
            tc.tile_pool(name="td", bufs=2) as tdpool,
            tc.tile_pool(name="xb", bufs=2) as xbpool,
            tc.tile_pool(name="xts", bufs=2) as xtspool,
            tc.tile_pool(name="hs", bufs=4) as hspool,
            tc.tile_pool(name="xtp", bufs=2, space="PSUM") as xtppool,
            tc.tile_pool(name="mp", bufs=4, space="PSUM") as mppool,
        ):
            idx_t = cpool.tile([128, IDX_COLS], i16)
            # slice the upload so the first gathers start early; alternate the
            # two HWDGE rings (sync + scalar) to halve the upload wall time
            idx_slice = 2 * CW_STRIDE
            for k in range(IDX_COLS // idx_slice):
                lo = k * idx_slice
                eng = nc.sync if k % 2 == 0 else nc.scalar
                eng.dma_start(idx_t[:, lo : lo + idx_slice], idx[:, lo : lo + idx_slice])
            id_t = cpool.tile([128, 128], bf16)
            nc.sync.dma_start(id_t[:], identb[:])
            w1t_t = cpool.tile([128, 2, 32], f32)
            nc.sync.dma_start(w1t_t[:], w1t[:])
            b1_t = cpool.tile([32, 1], f32)
            nc.sync.dma_start(b1_t[:], b1[:])
            w2l_t = cpool.tile([32, 32], f32)
            nc.sync.dma_start(w2l_t[:], w2l[:])
            b2_t = cpool.tile([32, 1], f32)
            nc.sync.dma_start(b2_t[:], b2[:])
            w3l_t = cpool.tile([32, 1], f32)
            nc.sync.dma_start(w3l_t[:], w3l[:])
            b3_t = cpool.tile([1, 1], f32)
            nc.sync.dma_start(b3_t[:], b3[:])
            out_t = cpool.tile([1, B_CORE], f32)

            # hoist the num_idxs register: bass otherwise re-emits a MOVE
            # before every gather call (~0.4 us of GpSimd queue time each)
            nreg = nc.gpsimd.to_reg(CHUNK)

            qn = 0
            for t in range(NT):
                gs = []
                for half in range(2):
                    g = gpool.tile([128, S, E], bf16, tag="g")
                    col = (2 * t + half) * CW_STRIDE
                    nc.gpsimd.dma_gather(
                        g[:],
                        emb[BIAS:, :],
                        idx_t[:, col : col + CW],
                        CHUNK,
                        nreg,
                        E,
                        single_packet=False,
                        queue_num=qn % 4,
                    )
                    qn += 1
                    gs.append(g)
                # feature-sum tree on DVE (all bf16, 2x mode); per-call
                # subtrees so the critical path after the LAST gather of a
                # t-block is only b1+c+d+x
                bsum = tbpool.tile([128, 2, 8, E], bf16, tag="tb")
                for half in range(2):
                    nc.vector.tensor_tensor(
                        out=bsum[:, half, :, :],
                        in0=gs[half][:, 0:8, :],
                        in1=gs[half][:, 8:16, :],
                        op=ADD,
                    )
                csum = tcpool.tile([128, 8, E], bf16, tag="tc")
                nc.vector.tensor_tensor(
                    out=csum[:], in0=bsum[:, 0, :, :], in1=bsum[:, 1, :, :], op=ADD
                )
                dsum = tdpool.tile([128, 4, E], bf16, tag="td")
                nc.vector.tensor_tensor(
                    out=dsum[:], in0=csum[:, 0:4, :], in1=csum[:, 4:8, :], op=ADD
                )
                esum = xbpool.tile([128, 2, E], bf16, tag="xe")
                nc.vector.tensor_tensor(
                    out=esum[:], in0=dsum[:, 0:2, :], in1=dsum[:, 2:4, :], op=ADD
                )
                x = xbpool.tile([128, E], bf16, tag="xb")
                nc.vector.tensor_tensor(
                    out=x[:], in0=esum[:, 0, :], in1=esum[:, 1, :], op=ADD
                )
                # transpose x -> x^T (PSUM f32), copy to SBUF on ScalarE
                xt_p = xtppool.tile([128, 2, 128], bf16, tag="xtp")
                for h in range(2):
                    nc.tensor.transpose(
                        xt_p[:, h, :], x[:, 128 * h : 128 * (h + 1)], id_t[:]
                    )
                xt_sb = xtspool.tile([128, 2, 128], f32, tag="xts")
                nc.scalar.activation(xt_sb[:], xt_p[:], AF.Copy)
                # MLP
                h1p = mppool.tile([32, 128], f32, tag="mp")
                for h in range(2):
                    nc.tensor.matmul(
                        h1p[:],
                        lhsT=w1t_t[:, h, :],
                        rhs=xt_sb[:, h, :],
                        start=(h == 0),
                        stop=(h == 1),
                    )
                h1s = hspool.tile([32, 128], f32, tag="hs")
                nc.scalar.activation(h1s[:], h1p[:], AF.Relu, bias=b1_t[:])
                h2p = mppool.tile([32, 128], f32, tag="mp")
                nc.tensor.matmul(h2p[:], lhsT=w2l_t[:], rhs=h1s[:], start=True, stop=True)
                h2s = hspool.tile([32, 128], f32, tag="hs")
                nc.scalar.activation(h2s[:], h2p[:], AF.Relu, bias=b2_t[:])
                yp = mppool.tile([1, 128], f32, tag="mp")
                nc.tensor.matmul(yp[:], lhsT=w3l_t[:], rhs=h2s[:], start=True, stop=True)
                nc.scalar.activation(
                    out_t[:, 128 * t : 128 * (t + 1)], yp[:], AF.Tanh, bias=b3_t[:]
                )
            nc.sync.dma_start(out[:], out_t[:])
    nc.compile()
    return nc


def _get_nc():
    global _nc_cache
    if _nc_cache is None:
        _nc_cache = _build()
    return _nc_cache


def _prep_indices(shard: np.ndarray) -> np.ndarray:
    """[F, B_CORE] int -> [128, IDX_COLS] int16 device layout.

    Per t-block two calls of 2064 indices (features 0-15 / 16-31 for 128
    batches + 16 zero pads so the Q7 trailing-negative trim is a no-op).
    Position j = f_local*128 + b_in, wrapped [16, CW] column-major and
    replicated across the 8 Q7 core groups; call starts 64 B aligned.
    """
    arr = np.asarray(shard).reshape(F, NT, 128).astype(np.int64) - BIAS
    outa = np.zeros((128, IDX_COLS), np.int16)
    for t in range(NT):
        for half in range(2):
            flat = arr[16 * half : 16 * half + 16, t, :].reshape(-1)  # j = f*128+b
            lst = np.zeros(CHUNK, np.int64)
            lst[:CHUNK_REAL] = flat
            lay = lst.reshape(CW, 16).T  # [16, CW]
            col = (2 * t + half) * CW_STRIDE
            outa[:, col : col + CW] = np.tile(lay, (8, 1))
    return outa


def build_in_maps(inputs: dict) -> list[dict]:
    import ml_dtypes

    indices = np.asarray(inputs["indices"])
    emb = np.asarray(inputs["emb"], dtype=np.float32).astype(ml_dtypes.bfloat16)
    w1 = np.asarray(inputs["w1"], dtype=np.float32)
    b1 = np.asarray(inputs["b1"], dtype=np.float32)
    w2 = np.asarray(inputs["w2"], dtype=np.float32)
    b2 = np.asarray(inputs["b2"], dtype=np.float32)
    w3 = np.asarray(inputs["w3"], dtype=np.float32)
    b3 = np.asarray(inputs["b3"], dtype=np.float32)

    common = {
        "emb": np.ascontiguousarray(emb),
        "identb": np.eye(128, dtype=np.float32).astype(ml_dtypes.bfloat16),
        "w1t": np.ascontiguousarray(w1.T.reshape(2, 128, 32).transpose(1, 0, 2)),
        "b1": b1.reshape(32, 1),
        "w2l": np.ascontiguousarray(w2.T),
        "b2": b2.reshape(32, 1),
        "w3l": np.ascontiguousarray(w3.T),
        "b3": b3.reshape(1, 1),
    }
    in_maps = []
    for c in range(N_CORES):
        shard = indices[:, c * B_CORE : (c + 1) * B_CORE]
        in_maps.append({**common, "idx": _prep_indices(shard)})
    return in_maps


def kernel(**inputs) -> np.ndarray:
    from concourse.bass_utils import run_bass_kernel_spmd

    in_maps = build_in_maps(inputs)
    nc = _get_nc()
    res = run_bass_kernel_spmd(nc, in_maps, core_ids=list(range(N_CORES)))
    ys = [np.asarray(res.results[c]["out"]).reshape(B_CORE) for c in range(N_CORES)]
    return np.concatenate(ys).reshape(BATCH, 1).astype(np.float32)
